# revision 3
# baseline (speedup 1.0000x reference)
"""Trainium2 Bass kernel for nn_KnowledgeFusion (bf16 pipeline, round 2).

Math (b=8, H=W=32, d=o=256, n_obj=15, n=16 with appended mean-emb):
  embs_aug = concat([embs, mean(embs)])                  [b,16,256]
  mask     = rasterized boxes (rounded to PATCH_SIZE=2)  [b,16,1024] in {0,1}
  proj     = patches @ Wp                                [b,1024,256]
  inj      = embs_aug @ We                               [b,16,256]
  s[hw]    = sum_n mask[n,hw]   (>=1: image box row)
  out      = proj + (mask^T @ inj) / s[:,None]           [b,1024,256]

Sharding: data-parallel over batch; core c computes batch c. Computed
transposed, outT[o, hw] = Wp^T @ patchesT + inj^T @ maskN, maskN =
mask/s. All tensor data is bf16 (fp32 PSUM accumulation); measured
end-to-end rel err ~4e-3 vs the fp32 reference (gate 2e-2).

Critical-path decisions (see trace analysis in the session log):
 - hdr (loc + grid constants) is the first DMA on the sync HWDGE ring:
   tiny transfers stuck behind bulk packets cost ~2us in earlier
   versions. Wp rides with the patches blobs (pw0/pw1) so the 8
   projection matmuls are not gated on the We/embs blob.
 - s -> 1/s uses one ones-matmul (s broadcast to 16 partitions) plus
   reciprocal_approx_fast (~18-bit, exact enough vs bf16 rounding),
   replacing a 3-link indicator chain that ping-ponged engines.
 - mean embedding via two K=15 N=1 matmuls on natural-layout embs
   rows (stationary) instead of DVE reduces: keeps the vector queue,
   which is the bottleneck engine, free for the mask pipeline.
 - dummy matmuls on a memset tile keep the PE busy from t=0 so the HAM
   clock gate opens during the input DMA window, not mid-compute.
 - per-bank evacuation alternates vector/scalar; each output half is
   DMA'd as soon as its two banks are out.
"""

import sys

sys.path.insert(0, "/opt/trn_rl_repo")

import numpy as np
import ml_dtypes

import concourse.bass as bass
import concourse.bacc as bacc
import concourse.mybir as mybir
from concourse import tile
from concourse import bass_utils
from concourse.alu_op_type import AluOpType

B, H, W, D = 8, 32, 32, 256
NOBJ, N = 15, 16
HW = H * W
O = 256
FP = mybir.dt.float32
BF = mybir.dt.bfloat16
I32 = mybir.dt.int32
AF = mybir.ActivationFunctionType

# hdr columns (i32): loc[4] grid[32]
HDR = 36
# wb2 blob columns (bf16): We0 We1 eT0 eT1 embs0 embs1
WB2 = 2 * O + 2 * N + 2 * 128  # 800
# pw blobs (bf16): [Wp_half | pT_half]
PW = O + HW  # 1280

N_WARM = 13  # dummy matmuls warming the PE clock during input DMA


def _bcast(ap, free_dims):
    """AP with explicit free-dim [step, count] pairs (step 0 = broadcast)."""
    return bass.AP(ap.tensor, ap.offset, ap.ap[:1] + free_dims)


def build_nc(debug: bool = False):
    nc = bacc.Bacc("TRN2", target_bir_lowering=False, debug=debug, num_devices=B)

    hdr = nc.dram_tensor("hdr", [N, HDR], I32, kind="ExternalInput")
    wb2 = nc.dram_tensor("wb2", [128, WB2], BF, kind="ExternalInput")
    pw0 = nc.dram_tensor("pw0", [128, PW], BF, kind="ExternalInput")
    pw1 = nc.dram_tensor("pw1", [128, PW], BF, kind="ExternalInput")
    outT = nc.dram_tensor("outT", [O, HW], BF, kind="ExternalOutput")

    with tile.TileContext(nc) as tc:
        with (
            nc.allow_low_precision(reason="bf16 matmuls, fp32 PSUM accumulation"),
            tc.tile_pool(name="big", bufs=1) as big,
            tc.tile_pool(name="small", bufs=1) as small,
            tc.tile_pool(name="outp", bufs=2) as outp,
            tc.tile_pool(name="psM", bufs=4, space=bass.MemorySpace.PSUM) as psM,
            tc.tile_pool(name="psS", bufs=2, space=bass.MemorySpace.PSUM) as psS,
            tc.tile_pool(name="psI", bufs=1, space=bass.MemorySpace.PSUM) as psI,
            tc.tile_pool(name="psW", bufs=1, space=bass.MemorySpace.PSUM) as psW,
        ):
            # ---- input DMAs: hdr first on the sync ring, then wb2;
            # pw0 on the scalar ring, pw1 on gpsimd (SWDGE)
            hdr_sb = small.tile([N, HDR], I32)
            nc.sync.dma_start(hdr_sb[:], hdr[:])
            wb2_sb = big.tile([128, WB2], BF)
            nc.sync.dma_start(wb2_sb[:], wb2[:])
            pw_sb = [big.tile([128, PW], BF, name=f"pw{h}") for h in range(2)]
            nc.scalar.dma_start(pw_sb[0][:], pw0[:])
            nc.gpsimd.dma_start(pw_sb[1][:], pw1[:])

            We_sb = [wb2_sb[:, O * k : O * (k + 1)] for k in range(2)]
            eT_sb = [wb2_sb[:, 2 * O + N * k : 2 * O + N * (k + 1)] for k in range(2)]
            em_sb = [
                wb2_sb[0:NOBJ, 2 * O + 2 * N + 128 * k : 2 * O + 2 * N + 128 * (k + 1)]
                for k in range(2)
            ]
            Wp_sb = [pw_sb[h][:, 0:O] for h in range(2)]
            pT_sb = [pw_sb[h][:, O : O + HW] for h in range(2)]

            # ---- PE warm-up: dummy matmuls on a memset tile
            wtile = small.tile([128, 256], BF, tag="wtile")
            nc.vector.memset(wtile[:], 0.0)
            warm_ps = psW.tile([128, 256], FP, tag="psW", name="warm")
            for _ in range(N_WARM):
                nc.tensor.matmul(
                    warm_ps[:], wtile[:, 0:128], wtile[:], start=True, stop=True
                )

            # ---- constants
            ones16 = small.tile([N, N], BF)
            nc.vector.memset(ones16[:], 1.0)
            ones15 = small.tile([NOBJ, 1], BF)
            nc.vector.memset(ones15[:], 1.0 / NOBJ)
            grid_f = small.tile([N, 32], FP)
            nc.vector.tensor_copy(grid_f[:], hdr_sb[:, 4:36])

            # ---- boxes: round starts down / ends up to multiples of 2
            loc_sb = hdr_sb[:, 0:4]
            locm = small.tile([N, 4], I32)
            nc.vector.tensor_scalar(locm[:], loc_sb, 1, None, op0=AluOpType.bitwise_and)
            boxes_i = small.tile([N, 4], I32)
            nc.vector.tensor_tensor(boxes_i[:], loc_sb, locm[:], op=AluOpType.subtract)
            nc.vector.tensor_scalar_add(boxes_i[:, 2:4], boxes_i[:, 2:4], 2)
            boxes_f = small.tile([N, 4], FP)
            nc.vector.tensor_copy(boxes_f[:], boxes_i[:])

            # ---- row/col interval masks [16, 32] fp32
            rowm = small.tile([N, 32], FP)
            colm = small.tile([N, 32], FP)
            tmp = small.tile([N, 32], FP, tag="cmp_tmp")
            nc.vector.tensor_scalar(tmp[:], grid_f[:], boxes_f[:, 2:3], None, op0=AluOpType.is_lt)
            nc.vector.scalar_tensor_tensor(
                rowm[:], grid_f[:], boxes_f[:, 0:1], tmp[:], op0=AluOpType.is_ge, op1=AluOpType.mult
            )
            tmp2 = small.tile([N, 32], FP, tag="cmp_tmp2")
            nc.vector.tensor_scalar(tmp2[:], grid_f[:], boxes_f[:, 3:4], None, op0=AluOpType.is_lt)
            nc.vector.scalar_tensor_tensor(
                colm[:], grid_f[:], boxes_f[:, 1:2], tmp2[:], op0=AluOpType.is_ge, op1=AluOpType.mult
            )

            # ---- mask [16, 1024] bf16 via broadcast outer product, in
            # halves so the s-chain can start on half 0 early
            mask_sb = small.tile([N, HW], BF, tag="mask")
            for h in range(2):
                nc.vector.tensor_tensor(
                    _bcast(mask_sb[:, 512 * h : 512 * (h + 1)], [[W, 16], [1, W]]),
                    _bcast(rowm[:, 16 * h : 16 * h + 16], [[1, 16], [0, W]]),
                    _bcast(colm[:], [[0, 16], [1, W]]),
                    op=AluOpType.mult,
                )

            # ---- s broadcast to 16 partitions; recB = 1/s; maskN = mask/s
            psumS = [psS.tile([N, 512], FP, tag="psS", name=f"psS{h}") for h in range(2)]
            recB_sb = small.tile([N, HW], FP, tag="recB")
            maskN_sb = small.tile([N, HW], BF, tag="maskN")
            for h in range(2):
                nc.tensor.matmul(
                    psumS[h][:], ones16[:], mask_sb[:, 512 * h : 512 * (h + 1)],
                    start=True, stop=True,
                )
                nc.vector.reciprocal_approx_fast(
                    recB_sb[:, 512 * h : 512 * (h + 1)], psumS[h][:]
                )
                nc.vector.tensor_tensor(
                    maskN_sb[:, 512 * h : 512 * (h + 1)],
                    mask_sb[:, 512 * h : 512 * (h + 1)],
                    recB_sb[:, 512 * h : 512 * (h + 1)],
                    op=AluOpType.mult,
                )

            # ---- mean embedding into the spare 16th column of each eT
            # chunk via tiny K=15 N=1 matmuls (bank reused from psW pool)
            mean_ps = psW.tile([128, 2], FP, tag="psW", name="mean")
            for k in range(2):
                nc.tensor.matmul(
                    mean_ps[:, k : k + 1], em_sb[k], ones15[:], start=True, stop=True
                )
                nc.vector.tensor_copy(eT_sb[k][:, NOBJ : NOBJ + 1], mean_ps[:, k : k + 1])

            # ---- inj = embs_aug @ We -> [16, 256] bf16
            psumI = psI.tile([N, O], FP, tag="psI")
            nc.tensor.matmul(psumI[:], eT_sb[0][:], We_sb[0][:], start=True, stop=False)
            nc.tensor.matmul(psumI[:], eT_sb[1][:], We_sb[1][:], start=False, stop=True)
            inj_sb = small.tile([N, O], BF)
            nc.scalar.activation(inj_sb[:], psumI[:], AF.Copy)

            # ---- main projection matmuls, h0 for both o-chunks first
            # (pw0 lands before pw1); injection closes banks later
            psum = [[None, None], [None, None]]
            for oc in range(2):
                for hc in range(2):
                    psum[oc][hc] = psM.tile([128, 512], FP, tag="psM", name=f"psM{oc}{hc}")
            for h in range(2):
                for oc in range(2):
                    o0 = 128 * oc
                    for hc in range(2):
                        nc.tensor.matmul(
                            psum[oc][hc][:],
                            Wp_sb[h][:, o0 : o0 + 128],
                            pT_sb[h][:, 512 * hc : 512 * (hc + 1)],
                            start=(h == 0), stop=False,
                        )

            # ---- injection matmuls close each bank; evacuate + store
            o_sb = [outp.tile([128, HW], BF, tag="osb", name=f"osb{oc}") for oc in range(2)]
            for oc in range(2):
                o0 = 128 * oc
                for hc in range(2):
                    nc.tensor.matmul(
                        psum[oc][hc][:],
                        inj_sb[:, o0 : o0 + 128],
                        maskN_sb[:, 512 * hc : 512 * (hc + 1)],
                        start=False, stop=True,
                    )
                    if hc == 0:
                        nc.vector.tensor_copy(
                            o_sb[oc][:, 512 * hc : 512 * (hc + 1)], psum[oc][hc][:]
                        )
                    else:
                        nc.scalar.activation(
                            o_sb[oc][:, 512 * hc : 512 * (hc + 1)], psum[oc][hc][:], AF.Copy
                        )
                eng = nc.sync if oc == 0 else nc.scalar
                eng.dma_start(outT[o0 : o0 + 128, :], o_sb[oc][:])

    nc.compile()
    return nc


def make_in_maps(inputs):
    patches = np.asarray(inputs["patches"], dtype=np.float32)
    embs = np.asarray(inputs["embs"], dtype=np.float32)
    locations = np.asarray(inputs["locations"], dtype=np.int32)
    Wp = np.asarray(inputs["Wp"], dtype=np.float32)
    We = np.asarray(inputs["We"], dtype=np.float32)
    BF_NP = ml_dtypes.bfloat16

    img_box = np.array([[0, 0, H, W]], dtype=np.int32)
    hdr_common = np.zeros((N, HDR), dtype=np.int32)
    hdr_common[:, 4:36] = np.arange(32, dtype=np.int32)[None, :]

    wb2_common = np.zeros((128, WB2), dtype=BF_NP)
    wb2_common[:, 0:O] = We[0:128].astype(BF_NP)
    wb2_common[:, O : 2 * O] = We[128:256].astype(BF_NP)

    Wp_bf = Wp.astype(BF_NP)

    in_maps = []
    for b in range(B):
        hdrb = hdr_common.copy()
        hdrb[:, 0:4] = np.concatenate([locations[b], img_box], 0)
        eTb = embs[b].T.astype(BF_NP)  # [256, 15]
        wb2b = wb2_common.copy()
        wb2b[:, 2 * O : 2 * O + NOBJ] = eTb[0:128]
        wb2b[:, 2 * O + N : 2 * O + N + NOBJ] = eTb[128:256]
        em_bf = embs[b].astype(BF_NP)  # [15, 256]
        wb2b[0:NOBJ, 2 * O + 2 * N : 2 * O + 2 * N + 128] = em_bf[:, 0:128]
        wb2b[0:NOBJ, 2 * O + 2 * N + 128 : 2 * O + 2 * N + 256] = em_bf[:, 128:256]
        pTb = patches[b].reshape(HW, D).T.astype(BF_NP)  # [256, 1024]
        pwb = [
            np.concatenate([Wp_bf[128 * h : 128 * (h + 1)], pTb[128 * h : 128 * (h + 1)]], axis=1)
            for h in range(2)
        ]
        in_maps.append(
            {
                "hdr": np.ascontiguousarray(hdrb),
                "wb2": wb2b,
                "pw0": np.ascontiguousarray(pwb[0]),
                "pw1": np.ascontiguousarray(pwb[1]),
            }
        )
    return in_maps


_NC = None


def _get_nc():
    global _NC
    if _NC is None:
        _NC = build_nc(debug=False)
    return _NC


def run(inputs, trace: bool = False, **kwargs):
    nc = _get_nc()
    res = bass_utils.run_bass_kernel_spmd(
        nc, make_in_maps(inputs), core_ids=list(range(B)), trace=trace, **kwargs
    )
    full = np.stack(
        [res.results[b]["outT"].astype(np.float32).T for b in range(B)], axis=0
    )
    return np.ascontiguousarray(full), res


def kernel(**inputs) -> np.ndarray:
    full, _ = run(inputs, trace=False)
    return full


# revision 6
# speedup vs baseline: 1.3213x; 1.3213x over previous
"""Trainium2 Bass kernel for nn_KnowledgeFusion (bf16 pipeline, round 3).

Math (b=8, H=W=32, d=o=256, n_obj=15, n=16 with appended mean-emb):
  embs_aug = concat([embs, mean(embs)])                  [b,16,256]
  mask     = rasterized boxes (rounded to PATCH_SIZE=2)  [b,16,1024] in {0,1}
  proj     = patches @ Wp                                [b,1024,256]
  inj      = embs_aug @ We                               [b,16,256]
  s[hw]    = sum_n mask[n,hw]   (>=1: image box row)
  out      = proj + (mask^T @ inj) / s[:,None]           [b,1024,256]

Sharding: data-parallel over batch; core c computes batch c. Computed
transposed, outT[o, hw] = Wp^T @ patchesT + inj^T @ maskN, maskN =
mask/s. All tensor data is bf16 (fp32 PSUM accumulation); measured
end-to-end rel err ~5e-3 vs the fp32 reference (gate 2e-2).

Critical-path decisions (from trace analysis of earlier rounds):
 - all input DMAs on the sync HWDGE ring in dependency order (hdr tiny
   first, then Wp+patches blobs); the We/embs blob on gpsimd. The
   scalar ring is kept clear of inputs: the ACT table load occupies it
   for the first ~1.5us. Tiny transfers otherwise get stuck behind
   bulk packets (~+2us observed).
 - s -> 1/s via RECIPROCAL_APPROX_FAST emitted directly with a bf16
   destination (wrapper only allows fp32 out; the fp32 constraint is
   on the *input* bit layout), so the maskN multiply runs in the DVE's
   2x bf16 mode. The vector queue is the critical chain hdr->maskN;
   mask half 1 is built on gpsimd in parallel.
 - the appended mean embedding is computed after the fact from inj15
   (mean commutes with the linear We projection): two tensor-engine
   matmuls plus scalar copies, keeping the vector queue clear.
 - dummy matmuls on a memset tile keep the PE busy from t=0 so the HAM
   clock gate opens during the input DMA window, not mid-compute.
"""

import sys

sys.path.insert(0, "/opt/trn_rl_repo")

import numpy as np
import ml_dtypes

import concourse.bass as bass
import concourse.bacc as bacc
import concourse.mybir as mybir
from concourse import tile
from concourse import bass_utils
from concourse.alu_op_type import AluOpType
from concourse.dve_ops import RECIP_APPROX_FAST_CONSTS, RECIPROCAL_APPROX_FAST

B, H, W, D = 8, 32, 32, 256
NOBJ, N = 15, 16
HW = H * W
O = 256
FP = mybir.dt.float32
BF = mybir.dt.bfloat16
I32 = mybir.dt.int32
AF = mybir.ActivationFunctionType

# hdr columns (i32): loc[4] grid[32]
HDR = 36
# wb2 blob columns (bf16): We0 We1 eT0 eT1 (15 cols each)
WB2 = 2 * O + 2 * NOBJ  # 542
# pw blobs (bf16): [Wp_half | pT_half]
PW = O + HW  # 1280

N_WARM = 13  # dummy matmuls warming the PE clock during input DMA


def _bcast(ap, free_dims):
    """AP with explicit free-dim [step, count] pairs (step 0 = broadcast)."""
    return bass.AP(ap.tensor, ap.offset, ap.ap[:1] + free_dims)


def build_nc(debug: bool = False):
    nc = bacc.Bacc("TRN2", target_bir_lowering=False, debug=debug, num_devices=B)

    hdr = nc.dram_tensor("hdr", [N, HDR], I32, kind="ExternalInput")
    wb2 = nc.dram_tensor("wb2", [128, WB2], BF, kind="ExternalInput")
    pw0 = nc.dram_tensor("pw0", [128, PW], BF, kind="ExternalInput")
    pw1 = nc.dram_tensor("pw1", [128, PW], BF, kind="ExternalInput")
    outT = nc.dram_tensor("outT", [O, HW], BF, kind="ExternalOutput")

    with tile.TileContext(nc) as tc:
        with (
            nc.allow_low_precision(reason="bf16 matmuls, fp32 PSUM accumulation"),
            tc.tile_pool(name="big", bufs=1) as big,
            tc.tile_pool(name="small", bufs=1) as small,
            tc.tile_pool(name="outp", bufs=2) as outp,
            tc.tile_pool(name="psM", bufs=4, space=bass.MemorySpace.PSUM) as psM,
            tc.tile_pool(name="psS", bufs=2, space=bass.MemorySpace.PSUM) as psS,
            tc.tile_pool(name="psI", bufs=1, space=bass.MemorySpace.PSUM) as psI,
            tc.tile_pool(name="psW", bufs=1, space=bass.MemorySpace.PSUM) as psW,
        ):
            # ---- input DMAs: hdr -> pw0 -> pw1 FIFO on the sync ring;
            # wb2 (needed latest-but-one) on gpsimd SWDGE
            hdr_sb = small.tile([N, HDR], I32)
            nc.sync.dma_start(hdr_sb[:], hdr[:])
            pw_sb = [big.tile([128, PW], BF, name=f"pw{h}") for h in range(2)]
            nc.sync.dma_start(pw_sb[0][:], pw0[:])
            nc.sync.dma_start(pw_sb[1][:], pw1[:])
            wb2_sb = big.tile([128, WB2], BF)
            nc.gpsimd.dma_start(wb2_sb[:], wb2[:])

            We_sb = [wb2_sb[:, O * k : O * (k + 1)] for k in range(2)]
            eT_sb = [wb2_sb[:, 2 * O + NOBJ * k : 2 * O + NOBJ * (k + 1)] for k in range(2)]
            Wp_sb = [pw_sb[h][:, 0:O] for h in range(2)]
            pT_sb = [pw_sb[h][:, O : O + HW] for h in range(2)]

            # ---- PE warm-up: dummy matmuls on a memset tile
            wtile = small.tile([128, 256], BF, tag="wtile")
            nc.vector.memset(wtile[:], 0.0)
            warm_ps = psW.tile([128, 256], FP, tag="psW", name="warm")
            for _ in range(N_WARM):
                nc.tensor.matmul(
                    warm_ps[:], wtile[:, 0:128], wtile[:], start=True, stop=True
                )

            # ---- constants
            ones16 = small.tile([N, N], BF)
            nc.vector.memset(ones16[:], 1.0)
            grid_f = small.tile([N, 32], FP)
            nc.vector.tensor_copy(grid_f[:], hdr_sb[:, 4:36])
            # A = [I15 | ones/15]: augmenting matmul stationary appending
            # the mean row to inj15 (engines cannot address partition 15
            # directly: partition base must be 0 mod 32)
            rowid = small.tile([NOBJ, 1], I32)
            nc.gpsimd.iota(rowid[:], pattern=[[1, 1]], base=0, channel_multiplier=1)
            colid = small.tile([NOBJ, NOBJ], I32)
            nc.gpsimd.iota(colid[:], pattern=[[1, NOBJ]], base=0, channel_multiplier=0)
            Aaug = small.tile([NOBJ, N], BF)
            nc.vector.tensor_tensor(
                Aaug[:, 0:NOBJ], colid[:], _bcast(rowid[:], [[0, NOBJ]]),
                op=AluOpType.is_equal,
            )
            nc.vector.memset(Aaug[:, NOBJ:N], 1.0 / NOBJ)

            # ---- boxes: round starts down / ends up to multiples of 2
            loc_sb = hdr_sb[:, 0:4]
            locm = small.tile([N, 4], I32)
            nc.vector.tensor_scalar(locm[:], loc_sb, 1, None, op0=AluOpType.bitwise_and)
            boxes_i = small.tile([N, 4], I32)
            nc.vector.tensor_tensor(boxes_i[:], loc_sb, locm[:], op=AluOpType.subtract)
            nc.vector.tensor_scalar_add(boxes_i[:, 2:4], boxes_i[:, 2:4], 2)
            boxes_f = small.tile([N, 4], FP)
            nc.vector.tensor_copy(boxes_f[:], boxes_i[:])

            # ---- row/col interval masks [16, 32] fp32
            rowm = small.tile([N, 32], FP)
            colm = small.tile([N, 32], FP)
            tmp = small.tile([N, 32], FP, tag="cmp_tmp")
            nc.vector.tensor_scalar(tmp[:], grid_f[:], boxes_f[:, 2:3], None, op0=AluOpType.is_lt)
            nc.vector.scalar_tensor_tensor(
                rowm[:], grid_f[:], boxes_f[:, 0:1], tmp[:], op0=AluOpType.is_ge, op1=AluOpType.mult
            )
            tmp2 = small.tile([N, 32], FP, tag="cmp_tmp2")
            nc.vector.tensor_scalar(tmp2[:], grid_f[:], boxes_f[:, 3:4], None, op0=AluOpType.is_lt)
            nc.vector.scalar_tensor_tensor(
                colm[:], grid_f[:], boxes_f[:, 1:2], tmp2[:], op0=AluOpType.is_ge, op1=AluOpType.mult
            )

            # ---- mask [16, 1024] bf16 via broadcast outer product; half
            # 0 on the vector engine, half 1 on gpsimd in parallel
            mask_sb = small.tile([N, HW], BF, tag="mask")
            for h, eng in ((0, nc.vector), (1, nc.gpsimd)):
                eng.tensor_tensor(
                    _bcast(mask_sb[:, 512 * h : 512 * (h + 1)], [[W, 16], [1, W]]),
                    _bcast(rowm[:, 16 * h : 16 * h + 16], [[1, 16], [0, W]]),
                    _bcast(colm[:], [[0, 16], [1, W]]),
                    op=AluOpType.mult,
                )

            # ---- s broadcast to 16 partitions; maskN = mask * recip(s)
            c = RECIP_APPROX_FAST_CONSTS
            psumS = [psS.tile([N, 512], FP, tag="psS", name=f"psS{h}") for h in range(2)]
            recB_sb = small.tile([N, HW], BF, tag="recB")
            maskN_sb = small.tile([N, HW], BF, tag="maskN")
            for h in range(2):
                nc.tensor.matmul(
                    psumS[h][:], ones16[:], mask_sb[:, 512 * h : 512 * (h + 1)],
                    start=True, stop=True,
                )
                nc.vector._custom_dve(
                    RECIPROCAL_APPROX_FAST,
                    out=recB_sb[:, 512 * h : 512 * (h + 1)],
                    in0=psumS[h][:],
                    s0=c["s0"], s1=c["s1"], imm2=c["imm2"],
                )
                nc.vector.tensor_tensor(
                    maskN_sb[:, 512 * h : 512 * (h + 1)],
                    mask_sb[:, 512 * h : 512 * (h + 1)],
                    recB_sb[:, 512 * h : 512 * (h + 1)],
                    op=AluOpType.mult,
                )

            # ---- main projection matmuls, h0 for both o-chunks first
            # (pw0 lands first); injection closes banks later
            psum = [[None, None], [None, None]]
            for oc in range(2):
                for hc in range(2):
                    psum[oc][hc] = psM.tile([128, 512], FP, tag="psM", name=f"psM{oc}{hc}")
            for h in range(2):
                for oc in range(2):
                    o0 = 128 * oc
                    for hc in range(2):
                        nc.tensor.matmul(
                            psum[oc][hc][:],
                            Wp_sb[h][:, o0 : o0 + 128],
                            pT_sb[h][:, 512 * hc : 512 * (hc + 1)],
                            start=(h == 0), stop=False,
                        )

            # ---- inj15 = embs @ We -> psum [15, 256]; appending the mean
            # row commutes with We: inj_aug = A^T @ inj15 via a tiny matmul
            inj15_sb = small.tile([NOBJ, O], BF)
            inj_sb = small.tile([N, O], BF)
            psumI = psI.tile([N, 512], FP, tag="psI")
            nc.tensor.matmul(
                psumI[0:NOBJ, 0:O], eT_sb[0][:], We_sb[0][:], start=True, stop=False
            )
            nc.tensor.matmul(
                psumI[0:NOBJ, 0:O], eT_sb[1][:], We_sb[1][:], start=False, stop=True
            )
            nc.scalar.activation(inj15_sb[:], psumI[0:NOBJ, 0:O], AF.Copy)
            nc.tensor.matmul(
                psumI[0:N, O : O + O], Aaug[:], inj15_sb[:], start=True, stop=True
            )
            nc.scalar.activation(inj_sb[:], psumI[0:N, O : O + O], AF.Copy)

            # ---- injection matmuls close each bank; evacuate + store
            o_sb = [outp.tile([128, HW], BF, tag="osb", name=f"osb{oc}") for oc in range(2)]
            for oc in range(2):
                o0 = 128 * oc
                for hc in range(2):
                    nc.tensor.matmul(
                        psum[oc][hc][:],
                        inj_sb[:, o0 : o0 + 128],
                        maskN_sb[:, 512 * hc : 512 * (hc + 1)],
                        start=False, stop=True,
                    )
                    if hc == 0:
                        nc.vector.tensor_copy(
                            o_sb[oc][:, 512 * hc : 512 * (hc + 1)], psum[oc][hc][:]
                        )
                    else:
                        nc.scalar.activation(
                            o_sb[oc][:, 512 * hc : 512 * (hc + 1)], psum[oc][hc][:], AF.Copy
                        )
                eng = nc.sync if oc == 0 else nc.scalar
                eng.dma_start(outT[o0 : o0 + 128, :], o_sb[oc][:])

    nc.compile()
    return nc


def make_in_maps(inputs):
    patches = np.asarray(inputs["patches"], dtype=np.float32)
    embs = np.asarray(inputs["embs"], dtype=np.float32)
    locations = np.asarray(inputs["locations"], dtype=np.int32)
    Wp = np.asarray(inputs["Wp"], dtype=np.float32)
    We = np.asarray(inputs["We"], dtype=np.float32)
    BF_NP = ml_dtypes.bfloat16

    img_box = np.array([[0, 0, H, W]], dtype=np.int32)
    hdr_common = np.zeros((N, HDR), dtype=np.int32)
    hdr_common[:, 4:36] = np.arange(32, dtype=np.int32)[None, :]

    wb2_common = np.zeros((128, WB2), dtype=BF_NP)
    wb2_common[:, 0:O] = We[0:128].astype(BF_NP)
    wb2_common[:, O : 2 * O] = We[128:256].astype(BF_NP)

    Wp_bf = Wp.astype(BF_NP)

    in_maps = []
    for b in range(B):
        hdrb = hdr_common.copy()
        hdrb[:, 0:4] = np.concatenate([locations[b], img_box], 0)
        eTb = embs[b].T.astype(BF_NP)  # [256, 15]
        wb2b = wb2_common.copy()
        wb2b[:, 2 * O : 2 * O + NOBJ] = eTb[0:128]
        wb2b[:, 2 * O + NOBJ : 2 * O + 2 * NOBJ] = eTb[128:256]
        pTb = patches[b].reshape(HW, D).T.astype(BF_NP)  # [256, 1024]
        pwb = [
            np.concatenate([Wp_bf[128 * h : 128 * (h + 1)], pTb[128 * h : 128 * (h + 1)]], axis=1)
            for h in range(2)
        ]
        in_maps.append(
            {
                "hdr": np.ascontiguousarray(hdrb),
                "wb2": wb2b,
                "pw0": np.ascontiguousarray(pwb[0]),
                "pw1": np.ascontiguousarray(pwb[1]),
            }
        )
    return in_maps


_NC = None


def _get_nc():
    global _NC
    if _NC is None:
        _NC = build_nc(debug=False)
    return _NC


def run(inputs, trace: bool = False, **kwargs):
    nc = _get_nc()
    res = bass_utils.run_bass_kernel_spmd(
        nc, make_in_maps(inputs), core_ids=list(range(B)), trace=trace, **kwargs
    )
    full = np.stack(
        [res.results[b]["outT"].astype(np.float32).T for b in range(B)], axis=0
    )
    return np.ascontiguousarray(full), res


def kernel(**inputs) -> np.ndarray:
    full, _ = run(inputs, trace=False)
    return full
